# revision 52
# baseline (speedup 1.0000x reference)
"""Bounded attention (per-head QK RMSNorm + RoPE + KV-cache attention) on 8
Trainium2 NeuronCores.

Sharding: data parallel over batch. B=16 batches -> 2 per core; each core runs
all 16 heads over its own KV cache slice, no cross-core communication.

Design v3 (mixed int8/fp8e3 KV stream, per-row scale folds; ~94us/core HBM
floor). The v1 baseline was DVE/ACT-bound: ~180us/core of int8->fp16 widening
across the two engines. v3 removes most of it:
  - K cache: half the chunks are fp8e3 (e3m4) consumed DIRECTLY by the PE as
    the mm1 stationary operand (mixed f8e3 x f16 matmul, verified exact on HW)
    -- zero widening cost; the other half stay int8 (widened on DVE) with
    per-row scales.
  - V cache: all int8 (V is the error-sensitive tensor). Most chunks widen on
    DVE/ACT as before; a couple ride the GPSIMD SWDGE cast-DMA (int8->fp16
    during the transfer, zero engine cost; bounded by the SBUF-write fabric).
  - Per (chunk, kv-row-pair) quantization scales, folded into the exp:
    activation computes exp(score * scale_vec + bias_vec) with per-partition
    (= per kv-row) vectors. scale_vec = softmax_scale / c_k_row folds the K
    scales; bias_vec = -ln(c_v_row) folds the V scales into the probabilities
    (the V row scale c_v_row is a small INTEGER, and the V tiles' extra
    "denominator" column holds exactly c_v_row, so the softmax denominator
    column stays exact). Per-row scales nearly halve int8 error vs per-head
    (8.8e-3 vs 1.42e-2 emulated), paying for the e3m4 chunks' extra noise.
  - Everything else keeps the v1 structure: q/k preprocessed on-device
    (rmsnorm+rope, fp32 -> fp16, PE transpose), 16x mm1 per 128-row tile with
    K stationary, one 128-col exp per chunk, 16x mm2 with expT stationary and
    V (+denom col) moving, PSUM strip accumulators, causal 4x4 corner,
    reciprocal+scale drain.
"""
import math
import numpy as np
import ml_dtypes

import concourse.bass as bass
import concourse.tile as tile
from concourse import bacc, mybir
from concourse.bass_utils import run_bass_kernel_spmd

F32 = mybir.dt.float32
F16 = mybir.dt.float16
F8E3 = mybir.dt.float8e3
I8 = mybir.dt.int8
AF = mybir.ActivationFunctionType
E3M4 = ml_dtypes.float8_e3m4

B, S, DIM = 16, 4, 2048
H, D = 16, 128
KV = 4096
EPS = 1e-5
N_CORES = 8
B_LOC = B // N_CORES  # 2
NT = KV // 128  # 32 tiles of 128 kv rows
NI = NT // 2  # 16 chunks of 256 kv rows per batch
SCALE = 1.0 / math.sqrt(D)
P = B_LOC * H * S  # 128 partitions in the (b, h, s) preproc layout
E = D + 1  # 129 = v columns + denominator column
E3K_SCALE = 2.0  # global scale for e3m4 K chunks (exact in fp32)

# per-chunk class assignment (same for both batches):
#   K: 'W' = int8 widened on DVE, 'E' = fp8e3 direct to PE
#   V: 'W' = int8 widened on DVE+ACT, 'C' = int8 cast-DMA via gpsimd
#       (C tiles are prefetched at kernel start so SWDGE latency never
#        stalls the accumulation chain)
KCLS = 'E' * NI
VCLS = 'W' * NI
KW_IDX = {i: n for n, i in enumerate(i for i in range(NI) if KCLS[i] == 'W')}
KE_IDX = {i: n for n, i in enumerate(i for i in range(NI) if KCLS[i] == 'E')}
VW_IDX = {i: n for n, i in enumerate(i for i in range(NI) if VCLS[i] == 'W')}
VC_IDX = {i: n for n, i in enumerate(i for i in range(NI) if VCLS[i] == 'C')}
NKW, NKE = len(KW_IDX), len(KE_IDX)
NVW, NVC = len(VW_IDX), len(VC_IDX)

VSPLIT = 928  # V widen split: ACT takes cols [0, VSPLIT), DVE the tail


def _col(b, h):
    # column offset of (b, h)'s four queries in the qT/kTn layouts
    return b * (H * S) + h * S


def _preprocess(nc, sb, pp, ps_pool, x_sb, w_sb, cos_sb, sin_sb, ident,
                eps_sb, name, ve):
    """rmsnorm + rope; returns [d, (b,h,s)] fp16 tile.

    x_sb is the pre-loaded [P, D] input tile (DMA issued early by the
    caller). `ve` selects the elementwise engine (nc.vector / nc.gpsimd)
    so the q and k chains run on different engines in parallel. rsqrt is
    computed by Newton iteration from z0=1 (the mean square of 128
    unit-normal samples concentrates near 1, so three iterations reach
    ~1e-7), avoiding the Sqrt activation-table set -- the Scalar engine
    then only ever loads the Exp set.
    """
    sq = pp.tile([P, D], F32, tag=f"{name}_sq")
    ssq = pp.tile([P, 1], F32, tag=f"{name}_ssq")
    nc.scalar.activation(sq[:], x_sb[:], AF.Square, accum_out=ssq[:])
    # m = ssq/D + eps; rsqrt(m) via Newton: z <- z*(1.5 - 0.5*m*z^2), z0=1
    m = pp.tile([P, 1], F32, tag=f"{name}_m")
    ve.tensor_scalar(m[:], ssq[:], 1.0 / D, EPS,
                     mybir.AluOpType.mult, mybir.AluOpType.add)
    hm = pp.tile([P, 1], F32, tag=f"{name}_hm")
    ve.tensor_scalar_mul(hm[:], m[:], -0.5)  # -m/2
    rinv = pp.tile([P, 1], F32, tag=f"{name}_rinv")
    # z1 = 1.5 - m/2  (z0 = 1)
    ve.tensor_scalar_add(rinv[:], hm[:], 1.5)
    t_a = pp.tile([P, 1], F32, tag=f"{name}_na")
    t_b = pp.tile([P, 1], F32, tag=f"{name}_nb")
    for _ in range(3):
        # z <- z * (1.5 + (-m/2) * z^2)
        ve.tensor_mul(t_a[:], rinv[:], rinv[:])
        ve.tensor_mul(t_b[:], t_a[:], hm[:])
        ve.tensor_scalar_add(t_a[:], t_b[:], 1.5)
        ve.tensor_mul(rinv[:], rinv[:], t_a[:])
    xn = pp.tile([P, D], F32, tag=f"{name}_xn")
    ve.tensor_scalar_mul(xn[:], x_sb[:], rinv[:])
    xnw = pp.tile([P, D], F32, tag=f"{name}_xnw")
    ve.tensor_mul(xnw[:], xn[:], w_sb[:])

    # rope on even/odd interleaved pairs
    xv = xnw[:].rearrange("p (x two) -> p x two", two=2)
    a, bb = xv[:, :, 0], xv[:, :, 1]
    xr = pp.tile([P, D], F32, tag=f"{name}_xr")
    xrv = xr[:].rearrange("p (x two) -> p x two", two=2)
    t1 = pp.tile([P, D // 2], F32, tag=f"{name}_t1")
    t2 = pp.tile([P, D // 2], F32, tag=f"{name}_t2")
    ve.tensor_mul(t1[:], a, cos_sb[:])
    ve.tensor_mul(t2[:], bb, sin_sb[:])
    ve.tensor_sub(xrv[:, :, 0], t1[:], t2[:])
    t3 = pp.tile([P, D // 2], F32, tag=f"{name}_t1")
    t4 = pp.tile([P, D // 2], F32, tag=f"{name}_t2")
    ve.tensor_mul(t3[:], a, sin_sb[:])
    ve.tensor_mul(t4[:], bb, cos_sb[:])
    ve.tensor_add(xrv[:, :, 1], t3[:], t4[:])

    return xr


def _transpose_cast(nc, sb, ps_pool, xr, ident, name):
    """[P, D] fp32 -> [d, (b,h,s)] fp16 via PE transpose."""
    xT_ps = ps_pool.tile([128, 512], F32, tag="sT")
    nc.tensor.transpose(xT_ps[0:D, 0:P], xr[:], ident[:])
    xT = sb.tile([D, P], F16, tag=f"{name}_T")
    # PSUM read must be DVE (GpSimd has no PSUM port)
    nc.vector.tensor_copy(xT[:], xT_ps[0:D, 0:P])
    return xT


def build():
    nc = bacc.Bacc("TRN2", target_bir_lowering=False, debug=False,
                   num_devices=N_CORES)

    qp_d = nc.dram_tensor("qp", [P, D], F32, kind="ExternalInput").ap()
    kp_d = nc.dram_tensor("kp", [P, D], F32, kind="ExternalInput").ap()
    vna_d = nc.dram_tensor("vna", [B_LOC, S, H * E], F16,
                           kind="ExternalInput").ap()
    ktw_d = (nc.dram_tensor("ktw", [B_LOC, NKW, D, 2 * H * 128], I8,
                            kind="ExternalInput").ap() if NKW else None)
    kte_d = (nc.dram_tensor("kte", [B_LOC, NKE, D, 2 * H * 128], F8E3,
                            kind="ExternalInput").ap() if NKE else None)
    vbw_d = (nc.dram_tensor("vbw", [B_LOC, NVW, 128, 2 * H * E], I8,
                            kind="ExternalInput").ap() if NVW else None)
    vbc_d = (nc.dram_tensor("vbc", [B_LOC, NVC, 128, 2 * H * E], I8,
                            kind="ExternalInput").ap() if NVC else None)
    ssc_d = nc.dram_tensor("ssc", [128, B_LOC * NI], F32,
                           kind="ExternalInput").ap()
    bia_d = nc.dram_tensor("bia", [128, B_LOC * NI], F32,
                           kind="ExternalInput").ap()
    cos_d = nc.dram_tensor("cos_b", [P, D // 2], F32, kind="ExternalInput").ap()
    sin_d = nc.dram_tensor("sin_b", [P, D // 2], F32, kind="ExternalInput").ap()
    wq_d = nc.dram_tensor("wq_b", [P, D], F32, kind="ExternalInput").ap()
    wk_d = nc.dram_tensor("wk_b", [P, D], F32, kind="ExternalInput").ap()
    id_d = nc.dram_tensor("ident", [128, 128], F32, kind="ExternalInput").ap()
    mask_d = nc.dram_tensor("mask", [S, H * S], F16,
                            kind="ExternalInput").ap()
    out_d = nc.dram_tensor("out", [B_LOC, S, DIM], F32,
                           kind="ExternalOutput").ap()

    with tile.TileContext(nc) as tc:
        with (
            tc.tile_pool(name="consts", bufs=1) as consts,
            tc.tile_pool(name="pp", bufs=1) as pp,
            tc.tile_pool(name="sb", bufs=1) as sb,
            tc.tile_pool(name="krg8", bufs=3) as krg8,
            tc.tile_pool(name="krgE", bufs=6) as krgE,
            tc.tile_pool(name="krg", bufs=3) as krg,
            tc.tile_pool(name="vrg8", bufs=6) as vrg8,
            tc.tile_pool(name="vrg", bufs=5) as vrg,
            tc.tile_pool(name="vrgC", bufs=B_LOC * max(NVC, 1)) as vrgC,
            tc.tile_pool(name="expp", bufs=4) as expp,
            tc.tile_pool(name="vnew", bufs=1) as vnew,
            tc.tile_pool(name="drain", bufs=2) as drain,
            tc.tile_pool(name="ps", bufs=3, space=bass.MemorySpace.PSUM) as ps,
            tc.tile_pool(name="psacc", bufs=1,
                         space=bass.MemorySpace.PSUM) as psacc,
        ):
            # inputs needed first (preprocessing critical path) go out on
            # the sync ring before anything else
            xq_sb = pp.tile([P, D], F32, tag="q_x")
            nc.sync.dma_start(xq_sb[:], qp_d)
            xk_sb = pp.tile([P, D], F32, tag="k_x")
            nc.sync.dma_start(xk_sb[:], kp_d)
            ident = consts.tile([128, 128], F32)
            nc.sync.dma_start(ident[:], id_d)

            # Prefetch all cast-DMA V tiles at kernel start: the SWDGE
            # (gpsimd) path has high latency, but these transfers are
            # independent of everything else, so issuing them first fully
            # hides it.
            chunks = [(b, i) for b in range(B_LOC) for i in range(NI)]
            cast_tiles = {}
            for bb in range(B_LOC):
                for ii in range(NI):
                    if VCLS[ii] == 'C':
                        vtf = vrgC.tile([128, 2 * H * E], F16, tag="vtfc",
                                        name=f"vtfc_{bb}_{ii}")
                        nc.gpsimd.dma_start(vtf[:], vbc_d[bb, VC_IDX[ii]])
                        cast_tiles[(bb, ii)] = vtf

            # Software pipeline over the 32 (b, i) chunks: DMAs issue 3
            # chunks ahead and int8->fp16 widening 1 chunk ahead of compute.
            dma_tiles = {}
            cvt_tiles = {}

            kt_tiles = {}
            vt_tiles = {}

            def issue_dma_k(idx):
                if idx >= len(chunks) or idx in kt_tiles:
                    return
                bb, ii = chunks[idx]
                if KCLS[ii] == 'W':
                    kt8 = krg8.tile([128, 2 * H * 128], I8, tag="kt8",
                                    name=f"kt8_{idx}")
                    nc.sync.dma_start(kt8[:], ktw_d[bb, KW_IDX[ii]])
                    kt_tiles[idx] = ('w', kt8)
                else:
                    kte = krgE.tile([128, 2 * H * 128], F8E3, tag="ktE",
                                    name=f"ktE_{idx}")
                    nc.sync.dma_start(kte[:], kte_d[bb, KE_IDX[ii]])
                    kt_tiles[idx] = ('e', kte)

            def issue_dma_v(idx):
                if idx >= len(chunks) or idx in vt_tiles:
                    return
                bb, ii = chunks[idx]
                if VCLS[ii] == 'W':
                    # V rides the ACT HWDGE ring so the two streams run on
                    # parallel DMA queues (one ring alone tops out ~320GB/s)
                    vt8 = vrg8.tile([128, 2 * H * E], I8, tag="vt8",
                                    name=f"vt8_{idx}")
                    nc.scalar.dma_start(vt8[:], vbw_d[bb, VW_IDX[ii]])
                    vt_tiles[idx] = ('w', vt8)
                else:
                    vt_tiles[idx] = ('c', cast_tiles[(bb, ii)])

            def issue_dma(idx):
                issue_dma_k(idx)
                issue_dma_v(idx)

            def issue_cvt(idx):
                # widen the int8 parts: V split ACT head / DVE tail
                if idx >= len(chunks):
                    return
                kcls, kt = kt_tiles.pop(idx)
                vcls, vt = vt_tiles.pop(idx)
                if kcls == 'w':
                    ktf = krg.tile([128, 2 * H * 128], F16, tag="ktf",
                                   name=f"ktf_{idx}")
                    nc.vector.tensor_copy(ktf[:], kt[:])
                    kt = ktf
                if vcls == 'w':
                    vtf = vrg.tile([128, 2 * H * E], F16, tag="vtf",
                                   name=f"vtf_{idx}")
                    nc.scalar.copy(vtf[:, 0:VSPLIT], vt[:, 0:VSPLIT])
                    nc.vector.tensor_copy(vtf[:, VSPLIT:2 * H * E],
                                          vt[:, VSPLIT:2 * H * E])
                    vt = vtf
                cvt_tiles[idx] = (kt, vt)

            # preproc consts ride the sync ring ahead of the stream (the
            # scalar ring's triggers sit in the ACT queue and would delay
            # the Squares; and the rope waits on cos/sin, so they must not
            # queue behind 512KB stream transfers)
            cos_sb = consts.tile([P, D // 2], F32)
            nc.sync.dma_start(cos_sb[:], cos_d)
            sin_sb = consts.tile([P, D // 2], F32)
            nc.sync.dma_start(sin_sb[:], sin_d)
            wq_sb = consts.tile([P, D], F32)
            nc.sync.dma_start(wq_sb[:], wq_d)
            wk_sb = consts.tile([P, D], F32)
            nc.sync.dma_start(wk_sb[:], wk_d)
            eps_sb = consts.tile([P, 1], F32)
            nc.vector.memset(eps_sb[:], EPS)

            issue_dma(0)
            issue_dma(1)

            xr_q = _preprocess(nc, sb, pp, ps, xq_sb, wq_sb, cos_sb, sin_sb,
                               ident, eps_sb, "q", nc.vector)
            xr_k = _preprocess(nc, sb, pp, ps, xk_sb, wk_sb, cos_sb, sin_sb,
                               ident, eps_sb, "k", nc.gpsimd)
            qT = _transpose_cast(nc, sb, ps, xr_q, ident, "q")
            # kTn's transpose is deferred to the first corner block so the
            # PE FIFO never blocks chunk-0 matmuls on the slower k chain
            kTn = None
            # third prefetch chunk goes out only now -- its trigger would
            # otherwise sit in the ACT queue ahead of the Squares and stall
            # them on the DMA-outstanding window. The sync ring carries only
            # DMA triggers, so K prefetches one chunk deeper than V.
            issue_dma(2)
            issue_dma_k(3)
            # prime the Exp activation-table set right after the last Square
            # use, so the ~2.7us load happens during the first chunk's DMA
            # wait rather than on the critical path before the first exp
            dummy = consts.tile([P, 1], F32)
            nc.scalar.activation(dummy[:], eps_sb[:], AF.Exp)

            # consts only needed by the exp / end-of-batch corner
            ssc_sb = consts.tile([128, B_LOC * NI], F32)
            nc.scalar.dma_start(ssc_sb[:], ssc_d)
            bia_sb = consts.tile([128, B_LOC * NI], F32)
            nc.scalar.dma_start(bia_sb[:], bia_d)
            mask16 = consts.tile([S, H * S], F16)
            nc.scalar.dma_start(mask16[:], mask_d)
            vnafs = []
            for b in range(B_LOC):
                vnaf = vnew.tile([S, H * E], F16, tag=f"vnaf{b}",
                                 name=f"vnaf_{b}")
                nc.scalar.dma_start(vnaf[:], vna_d[b])
                vnafs.append(vnaf)

            issue_cvt(0)
            accs = None
            # one-chunk software pipeline: mm2 for chunk c is emitted after
            # mm1 of chunk c+1, so the PE never stalls on the exp (ACT)
            pend = []

            def flush_pend():
                if not pend:
                    return
                pb, pi, pexpT, pvtf = pend.pop()

                def mm2(tt, j, stop):
                    v0 = tt * H * E + j * E
                    nc.tensor.matmul(
                        accs[j // 4][32 * (j % 4):32 * (j % 4) + 4, 0:E],
                        pexpT[:, tt * H * S + 4 * j:
                              tt * H * S + 4 * j + 4],
                        pvtf[:, v0:v0 + E],
                        start=False, stop=stop,
                        skip_group_check=True,
                        tile_position=(0, 32 * (j % 4)))

                if pi < NI - 1:
                    for tt in range(2):
                        for j in range(H):
                            mm2(tt, j, False)
                else:
                    # final chunk of batch pb: drain each accumulator bank
                    # right after its own last matmul so normalization and
                    # stores overlap the remaining banks' matmuls.
                    for j in range(H):
                        mm2(0, j, False)
                    o_all = drain.tile([128, 512], F32, tag="o_all",
                                       name=f"o_all_{pb}")
                    for g in range(4):
                        for j in range(4 * g, 4 * g + 4):
                            mm2(1, j, j % 4 == 3)
                        rs = drain.tile([128, 1], F32, tag=f"rs{g}",
                                        name=f"rs{g}_{pb}")
                        nc.vector.reciprocal(rs[:], accs[g][:, D:E])
                        nc.scalar.activation(o_all[:, g * D:(g + 1) * D],
                                             accs[g][:, 0:D], AF.Copy,
                                             scale=rs[:])
                    # stores ride the SWDGE (gpsimd) path: their scattered
                    # descriptors would otherwise sit in the sync ring's
                    # outstanding window and stall the K stream right at
                    # the batch boundary
                    for j in range(4):
                        nc.gpsimd.dma_start(
                            out_d[pb, :, :].rearrange(
                                "s (g j d) -> j s g d", g=4, d=D)[j],
                            o_all[32 * j:32 * j + S, :]
                            .rearrange("p (g d) -> p g d", d=D),
                        )

            for idx, (b, i) in enumerate(chunks):
                if i == 0:
                    # flush the previous batch's pending mm2 + drain BEFORE
                    # recycling the accumulator banks (PE is strict FIFO; a
                    # later-emitted matmul can't unblock an earlier one)
                    flush_pend()
                    # 4 PSUM accumulator banks (one per group of 4 heads):
                    # rows 32j+0..4 = o[q, :] of head 4g+j; col 128 = sums.
                    accs = [psacc.tile([128, 512], F32, tag=f"acc{g}",
                                       name=f"acc{g}_{b}")
                            for g in range(4)]
                    for g in range(4):
                        nc.vector.memset(accs[g][:, 0:E], 0.0)

                if i == 1:
                    # the 4 new (current) keys, causal-masked -- emitted
                    # here (not at i==0) so it never gates the first
                    # chunk's matmuls, and never lands in the kernel tail
                    if kTn is None:
                        kTn = _transpose_cast(nc, sb, ps, xr_k, ident, "k")
                    vnaf = vnafs[b]
                    sn = ps.tile([128, 512], F32, tag="sT",
                                 name=f"sn_{b}")
                    for j in range(H):
                        c = _col(b, j)
                        nc.tensor.matmul(sn[0:S, 4 * j:4 * j + 4],
                                         kTn[:, c:c + S], qT[:, c:c + S],
                                         start=(j == 0), stop=(j == H - 1),
                                         skip_group_check=True)
                    en = expp.tile([S, H * S], F16, tag="en",
                                   name=f"en_{b}")
                    nc.scalar.activation(en[:], sn[0:S, 0:H * S], AF.Exp,
                                         scale=SCALE)
                    enm = expp.tile([S, H * S], F16, tag="enm",
                                    name=f"enm_{b}")
                    nc.vector.tensor_mul(enm[:], en[:], mask16[:])
                    corner = (enm, vnafs[b])

                issue_dma_k(idx + 4)
                issue_dma_v(idx + 3)
                issue_cvt(idx + 1)
                ktf, vtf = cvt_tiles.pop(idx)
                # scores for both 128-row sub-tiles share one PSUM bank
                # -> a single 128-col exp per chunk
                sT = ps.tile([128, 512], F32, tag="sT", name=f"sT_{idx}")
                for tt in range(2):
                    for j in range(H):
                        c = _col(b, j)
                        k0 = tt * H * 128 + j * 128
                        nc.tensor.matmul(
                            sT[:, tt * H * S + 4 * j:
                               tt * H * S + 4 * j + 4],
                            ktf[:, k0:k0 + 128], qT[:, c:c + S],
                            start=(tt == 0 and j == 0),
                            stop=(tt == 1 and j == H - 1),
                            skip_group_check=True)
                expT = expp.tile([128, 2 * H * S], F16, tag="expT",
                                 name=f"expT_{idx}")
                scol = b * NI + i
                nc.scalar.activation(expT[:], sT[:, 0:2 * H * S], AF.Exp,
                                     scale=ssc_sb[:, scol:scol + 1],
                                     bias=bia_sb[:, scol:scol + 1])
                flush_pend()
                if i == 1:
                    # corner accumulation, emitted after a full chunk of PE
                    # work so the PE never stalls on the corner's exp
                    cenm, cvnaf = corner
                    for j in range(H):
                        nc.tensor.matmul(
                            accs[j // 4][32 * (j % 4):32 * (j % 4) + 4,
                                         0:E],
                            cenm[:, 4 * j:4 * j + 4],
                            cvnaf[:, j * E:(j + 1) * E],
                            start=False, stop=False,
                            skip_group_check=True,
                            tile_position=(0, 32 * (j % 4)))
                pend.append((b, i, expT, vtf))
            flush_pend()

    nc.compile()
    return nc


_NC_CACHE = []


def _get_nc():
    if not _NC_CACHE:
        _NC_CACHE.append(build())
    return _NC_CACHE[0]


def _make_in_maps(q, k, v, freqs_cos, freqs_sin, cache_k, cache_v, q_norm_w,
                  k_norm_w):
    q = np.asarray(q, dtype=np.float32)
    k = np.asarray(k, dtype=np.float32)
    v = np.asarray(v, dtype=np.float32)
    cache_k = np.asarray(cache_k, dtype=np.float32)
    cache_v = np.asarray(cache_v, dtype=np.float32)
    freqs_cos = np.asarray(freqs_cos, dtype=np.float32)
    freqs_sin = np.asarray(freqs_sin, dtype=np.float32)
    q_norm_w = np.asarray(q_norm_w, dtype=np.float32)
    k_norm_w = np.asarray(k_norm_w, dtype=np.float32)

    # host-side constant marshalling (layout/dtype helpers only)
    cos_b = np.ascontiguousarray(
        np.broadcast_to(freqs_cos[None, None], (B_LOC, H, S, D // 2))
        .reshape(P, D // 2))
    sin_b = np.ascontiguousarray(
        np.broadcast_to(freqs_sin[None, None], (B_LOC, H, S, D // 2))
        .reshape(P, D // 2))
    wq_b = np.ascontiguousarray(np.broadcast_to(q_norm_w[None, :], (P, D)))
    wk_b = np.ascontiguousarray(np.broadcast_to(k_norm_w[None, :], (P, D)))
    ident = np.eye(128, dtype=np.float32)
    # mask[t, j*4+i] = 1 if query i attends new key t (i >= t), per 16 heads
    mask = (np.arange(S)[None, :] >= np.arange(S)[:, None]).astype(np.float16)
    mask = np.ascontiguousarray(np.tile(mask, (1, H)))  # [4, 64]

    # q/k packed into the [(b h s), d] preproc layout
    qp_all = np.ascontiguousarray(
        q.reshape(B, S, H, D).transpose(0, 2, 1, 3)).reshape(B, H * S, D)
    kp_all = np.ascontiguousarray(
        k.reshape(B, S, H, D).transpose(0, 2, 1, 3)).reshape(B, H * S, D)

    # --- K cache: per (b, chunk, row-pair) scales -------------------------
    # chunk rows: [2(tt), 128(r), H, D]; scale shared across (tt, h, d)
    kc = cache_k.reshape(B, NI, 2, 128, H, D)
    k_abs = np.abs(kc).max(axis=(2, 4, 5))  # [B, NI, 128]
    k_abs = np.maximum(k_abs, 1e-6)
    ckr = (127.0 / k_abs).astype(np.float32)  # [B, NI, 128]
    ktw_all = np.empty((B, NKW, D, 2 * H * 128), np.int8)
    kte_all = np.empty((B, NKE, D, 2 * H * 128), E3M4)
    ssc_all = np.empty((B, NI, 128), np.float32)
    for i in range(NI):
        blk = kc[:, i]  # [B, 2, 128, H, D]
        if KCLS[i] == 'W':
            ki = np.clip(np.round(blk * ckr[:, i, None, :, None, None]),
                         -127, 127).astype(np.int8)
            # -> [B, D, 2, H, 128] -> [B, D, 2*H*128]
            ktw_all[:, KW_IDX[i]] = (ki.transpose(0, 4, 1, 3, 2)
                                     .reshape(B, D, 2 * H * 128))
            ssc_all[:, i] = SCALE / ckr[:, i]
        else:
            ke = (blk * E3K_SCALE).astype(E3M4)
            kte_all[:, KE_IDX[i]] = (ke.transpose(0, 4, 1, 3, 2)
                                     .reshape(B, D, 2 * H * 128))
            ssc_all[:, i] = SCALE / E3K_SCALE

    # --- V cache: per (b, chunk, row-pair) INTEGER scales -----------------
    vc = cache_v.reshape(B, NI, 2, 128, H, D)
    v_abs = np.abs(vc).max(axis=(2, 4, 5))  # [B, NI, 128]
    v_abs = np.maximum(v_abs, 1e-6)
    cvr = np.clip(np.floor(127.0 / v_abs), 1, 127).astype(np.float32)
    vbw_all = np.empty((B, NVW, 128, 2 * H * E), np.int8)
    vbc_all = np.empty((B, NVC, 128, 2 * H * E), np.int8)
    bia_all = np.empty((B, NI, 128), np.float32)
    vaug = np.empty((B, 2, 128, H, E), np.int8)
    for i in range(NI):
        blk = vc[:, i]  # [B, 2, 128, H, D]
        vi = np.clip(np.round(blk * cvr[:, i, None, :, None, None]),
                     -127, 127).astype(np.int8)
        vaug[:, :, :, :, 0:D] = vi
        vaug[:, :, :, :, D] = cvr[:, i, None, :, None]
        # [B, 2, 128, H, E] -> [B, 128, 2, H, E] -> [B, 128, 2*H*E]
        packed = (vaug.transpose(0, 2, 1, 3, 4).reshape(B, 128, 2 * H * E))
        if VCLS[i] == 'W':
            vbw_all[:, VW_IDX[i]] = packed
        else:
            vbc_all[:, VC_IDX[i]] = packed
        bia_all[:, i] = -np.log(cvr[:, i])

    # new-token V with the exact ones column (scale 1)
    vna_all = np.empty((B, S, H, E), np.float16)
    vna_all[:, :, :, 0:D] = v.reshape(B, S, H, D)
    vna_all[:, :, :, D] = 1.0

    in_maps = []
    for ci in range(N_CORES):
        bs = slice(ci * B_LOC, (ci + 1) * B_LOC)
        # ssc/bia: [128(row), B_LOC*NI] with col = b*NI + i
        ssc = np.ascontiguousarray(
            ssc_all[bs].reshape(B_LOC * NI, 128).T)
        bia = np.ascontiguousarray(
            bia_all[bs].reshape(B_LOC * NI, 128).T)
        im = {
            "qp": np.ascontiguousarray(qp_all[bs]).reshape(P, D),
            "kp": np.ascontiguousarray(kp_all[bs]).reshape(P, D),
            "vna": np.ascontiguousarray(vna_all[bs]).reshape(B_LOC, S, H * E),
            "ssc": ssc, "bia": bia,
            "cos_b": cos_b, "sin_b": sin_b, "wq_b": wq_b, "wk_b": wk_b,
            "ident": ident, "mask": mask,
        }
        if NKW:
            im["ktw"] = np.ascontiguousarray(ktw_all[bs])
        if NKE:
            im["kte"] = np.ascontiguousarray(kte_all[bs])
        if NVW:
            im["vbw"] = np.ascontiguousarray(vbw_all[bs])
        if NVC:
            im["vbc"] = np.ascontiguousarray(vbc_all[bs])
        in_maps.append(im)
    return in_maps


def run(q, k, v, freqs_cos, freqs_sin, cache_k, cache_v, q_norm_w, k_norm_w,
        trace=False):
    in_maps = _make_in_maps(q, k, v, freqs_cos, freqs_sin, cache_k, cache_v,
                            q_norm_w, k_norm_w)
    nc = _get_nc()
    res = run_bass_kernel_spmd(nc, in_maps, list(range(N_CORES)), trace=trace)
    out = np.concatenate([res.results[i]["out"] for i in range(N_CORES)],
                         axis=0)
    return out.reshape(B, S, DIM), res


def kernel(q, k, v, freqs_cos, freqs_sin, cache_k, cache_v, q_norm_w,
           k_norm_w):
    out, _ = run(q, k, v, freqs_cos, freqs_sin, cache_k, cache_v, q_norm_w,
                 k_norm_w)
    return out


# revision 54
# speedup vs baseline: 1.0575x; 1.0575x over previous
"""Bounded attention (per-head QK RMSNorm + RoPE + KV-cache attention) on 8
Trainium2 NeuronCores.

Sharding: data parallel over batch. B=16 batches -> 2 per core; each core runs
all 16 heads over its own KV cache slice, no cross-core communication.

Design v3 (mixed int8/fp8e3 KV stream, per-row scale folds; ~94us/core HBM
floor). The v1 baseline was DVE/ACT-bound: ~180us/core of int8->fp16 widening
across the two engines. v3 removes most of it:
  - K cache: half the chunks are fp8e3 (e3m4) consumed DIRECTLY by the PE as
    the mm1 stationary operand (mixed f8e3 x f16 matmul, verified exact on HW)
    -- zero widening cost; the other half stay int8 (widened on DVE) with
    per-row scales.
  - V cache: all int8 (V is the error-sensitive tensor). Most chunks widen on
    DVE/ACT as before; a couple ride the GPSIMD SWDGE cast-DMA (int8->fp16
    during the transfer, zero engine cost; bounded by the SBUF-write fabric).
  - Per (chunk, kv-row-pair) quantization scales, folded into the exp:
    activation computes exp(score * scale_vec + bias_vec) with per-partition
    (= per kv-row) vectors. scale_vec = softmax_scale / c_k_row folds the K
    scales; bias_vec = -ln(c_v_row) folds the V scales into the probabilities
    (the V row scale c_v_row is a small INTEGER, and the V tiles' extra
    "denominator" column holds exactly c_v_row, so the softmax denominator
    column stays exact). Per-row scales nearly halve int8 error vs per-head
    (8.8e-3 vs 1.42e-2 emulated), paying for the e3m4 chunks' extra noise.
  - Everything else keeps the v1 structure: q/k preprocessed on-device
    (rmsnorm+rope, fp32 -> fp16, PE transpose), 16x mm1 per 128-row tile with
    K stationary, one 128-col exp per chunk, 16x mm2 with expT stationary and
    V (+denom col) moving, PSUM strip accumulators, causal 4x4 corner,
    reciprocal+scale drain.
"""
import math
import numpy as np
import ml_dtypes

import concourse.bass as bass
import concourse.tile as tile
from concourse import bacc, mybir
from concourse.bass_utils import run_bass_kernel_spmd

F32 = mybir.dt.float32
F16 = mybir.dt.float16
F8E3 = mybir.dt.float8e3
I8 = mybir.dt.int8
AF = mybir.ActivationFunctionType
E3M4 = ml_dtypes.float8_e3m4

B, S, DIM = 16, 4, 2048
H, D = 16, 128
KV = 4096
EPS = 1e-5
N_CORES = 8
B_LOC = B // N_CORES  # 2
NT = KV // 128  # 32 tiles of 128 kv rows
NI = NT // 2  # 16 chunks of 256 kv rows per batch
SCALE = 1.0 / math.sqrt(D)
P = B_LOC * H * S  # 128 partitions in the (b, h, s) preproc layout
E = D + 1  # 129 = v columns + denominator column
E3K_SCALE = 2.0  # global scale for e3m4 K chunks (exact in fp32)

# per-chunk class assignment (same for both batches):
#   K: 'W' = int8 widened on DVE, 'E' = fp8e3 direct to PE
#   V: 'W' = int8 widened on DVE+ACT, 'C' = int8 cast-DMA via gpsimd
#       (C tiles are prefetched at kernel start so SWDGE latency never
#        stalls the accumulation chain)
KCLS = 'E' * NI
VCLS = 'W' * NI
KW_IDX = {i: n for n, i in enumerate(i for i in range(NI) if KCLS[i] == 'W')}
KE_IDX = {i: n for n, i in enumerate(i for i in range(NI) if KCLS[i] == 'E')}
VW_IDX = {i: n for n, i in enumerate(i for i in range(NI) if VCLS[i] == 'W')}
VC_IDX = {i: n for n, i in enumerate(i for i in range(NI) if VCLS[i] == 'C')}
NKW, NKE = len(KW_IDX), len(KE_IDX)
NVW, NVC = len(VW_IDX), len(VC_IDX)

VSPLIT = 928  # V widen split: ACT takes cols [0, VSPLIT), DVE the tail


def _col(b, h):
    # column offset of (b, h)'s four queries in the qT/kTn layouts
    return b * (H * S) + h * S


def _preprocess(nc, sb, pp, ps_pool, x_sb, w_sb, cos_sb, sin_sb, ident,
                eps_sb, name, ve):
    """rmsnorm + rope; returns [d, (b,h,s)] fp16 tile.

    x_sb is the pre-loaded [P, D] input tile (DMA issued early by the
    caller). `ve` selects the elementwise engine (nc.vector / nc.gpsimd)
    so the q and k chains run on different engines in parallel. rsqrt is
    computed by Newton iteration from z0=1 (the mean square of 128
    unit-normal samples concentrates near 1, so three iterations reach
    ~1e-7), avoiding the Sqrt activation-table set -- the Scalar engine
    then only ever loads the Exp set.
    """
    sq = pp.tile([P, D], F32, tag=f"{name}_sq")
    ssq = pp.tile([P, 1], F32, tag=f"{name}_ssq")
    nc.scalar.activation(sq[:], x_sb[:], AF.Square, accum_out=ssq[:])
    # m = ssq/D + eps; rsqrt(m) via Newton: z <- z*(1.5 - 0.5*m*z^2), z0=1
    m = pp.tile([P, 1], F32, tag=f"{name}_m")
    ve.tensor_scalar(m[:], ssq[:], 1.0 / D, EPS,
                     mybir.AluOpType.mult, mybir.AluOpType.add)
    hm = pp.tile([P, 1], F32, tag=f"{name}_hm")
    ve.tensor_scalar_mul(hm[:], m[:], -0.5)  # -m/2
    rinv = pp.tile([P, 1], F32, tag=f"{name}_rinv")
    # z1 = 1.5 - m/2  (z0 = 1)
    ve.tensor_scalar_add(rinv[:], hm[:], 1.5)
    t_a = pp.tile([P, 1], F32, tag=f"{name}_na")
    t_b = pp.tile([P, 1], F32, tag=f"{name}_nb")
    for _ in range(3):
        # z <- z * (1.5 + (-m/2) * z^2)
        ve.tensor_mul(t_a[:], rinv[:], rinv[:])
        ve.tensor_mul(t_b[:], t_a[:], hm[:])
        ve.tensor_scalar_add(t_a[:], t_b[:], 1.5)
        ve.tensor_mul(rinv[:], rinv[:], t_a[:])
    xn = pp.tile([P, D], F32, tag=f"{name}_xn")
    ve.tensor_scalar_mul(xn[:], x_sb[:], rinv[:])
    xnw = pp.tile([P, D], F32, tag=f"{name}_xnw")
    ve.tensor_mul(xnw[:], xn[:], w_sb[:])

    # rope on even/odd interleaved pairs
    xv = xnw[:].rearrange("p (x two) -> p x two", two=2)
    a, bb = xv[:, :, 0], xv[:, :, 1]
    xr = pp.tile([P, D], F32, tag=f"{name}_xr")
    xrv = xr[:].rearrange("p (x two) -> p x two", two=2)
    t1 = pp.tile([P, D // 2], F32, tag=f"{name}_t1")
    t2 = pp.tile([P, D // 2], F32, tag=f"{name}_t2")
    ve.tensor_mul(t1[:], a, cos_sb[:])
    ve.tensor_mul(t2[:], bb, sin_sb[:])
    ve.tensor_sub(xrv[:, :, 0], t1[:], t2[:])
    t3 = pp.tile([P, D // 2], F32, tag=f"{name}_t1")
    t4 = pp.tile([P, D // 2], F32, tag=f"{name}_t2")
    ve.tensor_mul(t3[:], a, sin_sb[:])
    ve.tensor_mul(t4[:], bb, cos_sb[:])
    ve.tensor_add(xrv[:, :, 1], t3[:], t4[:])

    return xr


def _transpose_cast(nc, sb, ps_pool, xr, ident, name):
    """[P, D] fp32 -> [d, (b,h,s)] fp16 via PE transpose."""
    xT_ps = ps_pool.tile([128, 512], F32, tag="sT")
    nc.tensor.transpose(xT_ps[0:D, 0:P], xr[:], ident[:])
    xT = sb.tile([D, P], F16, tag=f"{name}_T")
    # PSUM read must be DVE (GpSimd has no PSUM port)
    nc.vector.tensor_copy(xT[:], xT_ps[0:D, 0:P])
    return xT


def build():
    nc = bacc.Bacc("TRN2", target_bir_lowering=False, debug=False,
                   num_devices=N_CORES)

    qp_d = nc.dram_tensor("qp", [P, D], F32, kind="ExternalInput").ap()
    kp_d = nc.dram_tensor("kp", [P, D], F32, kind="ExternalInput").ap()
    vna_d = nc.dram_tensor("vna", [B_LOC, S, H * E], F16,
                           kind="ExternalInput").ap()
    ktw_d = (nc.dram_tensor("ktw", [B_LOC, NKW, D, 2 * H * 128], I8,
                            kind="ExternalInput").ap() if NKW else None)
    kte_d = (nc.dram_tensor("kte", [B_LOC, NKE, D, 2 * H * 128], F8E3,
                            kind="ExternalInput").ap() if NKE else None)
    vbw_d = (nc.dram_tensor("vbw", [B_LOC, NVW, 128, 2 * H * E], I8,
                            kind="ExternalInput").ap() if NVW else None)
    vbc_d = (nc.dram_tensor("vbc", [B_LOC, NVC, 128, 2 * H * E], I8,
                            kind="ExternalInput").ap() if NVC else None)
    ssc_d = nc.dram_tensor("ssc", [128, B_LOC * NI], F32,
                           kind="ExternalInput").ap()
    bia_d = nc.dram_tensor("bia", [128, B_LOC * NI], F32,
                           kind="ExternalInput").ap()
    cos_d = nc.dram_tensor("cos_b", [P, D // 2], F32, kind="ExternalInput").ap()
    sin_d = nc.dram_tensor("sin_b", [P, D // 2], F32, kind="ExternalInput").ap()
    wq_d = nc.dram_tensor("wq_b", [P, D], F32, kind="ExternalInput").ap()
    wk_d = nc.dram_tensor("wk_b", [P, D], F32, kind="ExternalInput").ap()
    id_d = nc.dram_tensor("ident", [128, 128], F32, kind="ExternalInput").ap()
    mask_d = nc.dram_tensor("mask", [S, H * S], F16,
                            kind="ExternalInput").ap()
    out_d = nc.dram_tensor("out", [B_LOC, S, DIM], F32,
                           kind="ExternalOutput").ap()

    with tile.TileContext(nc) as tc:
        with (
            tc.tile_pool(name="consts", bufs=1) as consts,
            tc.tile_pool(name="pp", bufs=1) as pp,
            tc.tile_pool(name="sb", bufs=1) as sb,
            tc.tile_pool(name="krg8", bufs=3) as krg8,
            tc.tile_pool(name="krgE", bufs=6) as krgE,
            tc.tile_pool(name="krg", bufs=3) as krg,
            tc.tile_pool(name="vrg8", bufs=6) as vrg8,
            tc.tile_pool(name="vrg", bufs=5) as vrg,
            tc.tile_pool(name="vrgC", bufs=B_LOC * max(NVC, 1)) as vrgC,
            tc.tile_pool(name="expp", bufs=4) as expp,
            tc.tile_pool(name="vnew", bufs=1) as vnew,
            tc.tile_pool(name="drain", bufs=2) as drain,
            tc.tile_pool(name="ps", bufs=3, space=bass.MemorySpace.PSUM) as ps,
            tc.tile_pool(name="psacc", bufs=1,
                         space=bass.MemorySpace.PSUM) as psacc,
        ):
            # inputs needed first (preprocessing critical path) go out on
            # the sync ring before anything else
            xq_sb = pp.tile([P, D], F32, tag="q_x")
            nc.sync.dma_start(xq_sb[:], qp_d)
            xk_sb = pp.tile([P, D], F32, tag="k_x")
            nc.sync.dma_start(xk_sb[:], kp_d)
            ident = consts.tile([128, 128], F32)
            nc.sync.dma_start(ident[:], id_d)

            # Prefetch all cast-DMA V tiles at kernel start: the SWDGE
            # (gpsimd) path has high latency, but these transfers are
            # independent of everything else, so issuing them first fully
            # hides it.
            chunks = [(b, i) for b in range(B_LOC) for i in range(NI)]
            cast_tiles = {}
            for bb in range(B_LOC):
                for ii in range(NI):
                    if VCLS[ii] == 'C':
                        vtf = vrgC.tile([128, 2 * H * E], F16, tag="vtfc",
                                        name=f"vtfc_{bb}_{ii}")
                        nc.gpsimd.dma_start(vtf[:], vbc_d[bb, VC_IDX[ii]])
                        cast_tiles[(bb, ii)] = vtf

            # Software pipeline over the 32 (b, i) chunks: DMAs issue 3
            # chunks ahead and int8->fp16 widening 1 chunk ahead of compute.
            dma_tiles = {}
            cvt_tiles = {}

            kt_tiles = {}
            vt_tiles = {}

            def issue_dma_k(idx):
                if idx >= len(chunks) or idx in kt_tiles:
                    return
                bb, ii = chunks[idx]
                if KCLS[ii] == 'W':
                    kt8 = krg8.tile([128, 2 * H * 128], I8, tag="kt8",
                                    name=f"kt8_{idx}")
                    nc.sync.dma_start(kt8[:], ktw_d[bb, KW_IDX[ii]])
                    kt_tiles[idx] = ('w', kt8)
                else:
                    kte = krgE.tile([128, 2 * H * 128], F8E3, tag="ktE",
                                    name=f"ktE_{idx}")
                    nc.sync.dma_start(kte[:], kte_d[bb, KE_IDX[ii]])
                    kt_tiles[idx] = ('e', kte)

            def issue_dma_v(idx):
                if idx >= len(chunks) or idx in vt_tiles:
                    return
                bb, ii = chunks[idx]
                if VCLS[ii] == 'W':
                    # V rides the ACT HWDGE ring so the two streams run on
                    # parallel DMA queues (one ring alone tops out ~320GB/s)
                    vt8 = vrg8.tile([128, 2 * H * E], I8, tag="vt8",
                                    name=f"vt8_{idx}")
                    nc.scalar.dma_start(vt8[:], vbw_d[bb, VW_IDX[ii]])
                    vt_tiles[idx] = ('w', vt8)
                else:
                    vt_tiles[idx] = ('c', cast_tiles[(bb, ii)])

            def issue_dma(idx):
                issue_dma_k(idx)
                issue_dma_v(idx)

            def issue_cvt(idx):
                # widen the int8 parts: V split ACT head / DVE tail
                if idx >= len(chunks):
                    return
                kcls, kt = kt_tiles.pop(idx)
                vcls, vt = vt_tiles.pop(idx)
                if kcls == 'w':
                    ktf = krg.tile([128, 2 * H * 128], F16, tag="ktf",
                                   name=f"ktf_{idx}")
                    nc.vector.tensor_copy(ktf[:], kt[:])
                    kt = ktf
                if vcls == 'w':
                    vtf = vrg.tile([128, 2 * H * E], F16, tag="vtf",
                                   name=f"vtf_{idx}")
                    nc.scalar.copy(vtf[:, 0:VSPLIT], vt[:, 0:VSPLIT])
                    nc.vector.tensor_copy(vtf[:, VSPLIT:2 * H * E],
                                          vt[:, VSPLIT:2 * H * E])
                    vt = vtf
                cvt_tiles[idx] = (kt, vt)

            # preproc consts ride the sync ring ahead of the stream (the
            # scalar ring's triggers sit in the ACT queue and would delay
            # the Squares; and the rope waits on cos/sin, so they must not
            # queue behind 512KB stream transfers)
            cos_sb = consts.tile([P, D // 2], F32)
            nc.sync.dma_start(cos_sb[:], cos_d)
            sin_sb = consts.tile([P, D // 2], F32)
            nc.sync.dma_start(sin_sb[:], sin_d)
            wq_sb = consts.tile([P, D], F32)
            nc.sync.dma_start(wq_sb[:], wq_d)
            wk_sb = consts.tile([P, D], F32)
            nc.sync.dma_start(wk_sb[:], wk_d)
            eps_sb = consts.tile([P, 1], F32)
            nc.vector.memset(eps_sb[:], EPS)

            issue_dma(0)
            issue_dma(1)

            xr_q = _preprocess(nc, sb, pp, ps, xq_sb, wq_sb, cos_sb, sin_sb,
                               ident, eps_sb, "q", nc.vector)
            xr_k = _preprocess(nc, sb, pp, ps, xk_sb, wk_sb, cos_sb, sin_sb,
                               ident, eps_sb, "k", nc.gpsimd)
            qT = _transpose_cast(nc, sb, ps, xr_q, ident, "q")
            # kTn's transpose is deferred to the first corner block so the
            # PE FIFO never blocks chunk-0 matmuls on the slower k chain
            kTn = None
            # third prefetch chunk goes out only now -- its trigger would
            # otherwise sit in the ACT queue ahead of the Squares and stall
            # them on the DMA-outstanding window
            issue_dma(2)
            # prime the Exp activation-table set right after the last Square
            # use, so the ~2.7us load happens during the first chunk's DMA
            # wait rather than on the critical path before the first exp
            dummy = consts.tile([P, 1], F32)
            nc.scalar.activation(dummy[:], eps_sb[:], AF.Exp)

            # consts only needed by the exp / end-of-batch corner
            ssc_sb = consts.tile([128, B_LOC * NI], F32)
            nc.scalar.dma_start(ssc_sb[:], ssc_d)
            bia_sb = consts.tile([128, B_LOC * NI], F32)
            nc.scalar.dma_start(bia_sb[:], bia_d)
            mask16 = consts.tile([S, H * S], F16)
            nc.scalar.dma_start(mask16[:], mask_d)
            vnafs = []
            for b in range(B_LOC):
                vnaf = vnew.tile([S, H * E], F16, tag=f"vnaf{b}",
                                 name=f"vnaf_{b}")
                nc.scalar.dma_start(vnaf[:], vna_d[b])
                vnafs.append(vnaf)

            issue_cvt(0)
            accs = None
            # one-chunk software pipeline: mm2 for chunk c is emitted after
            # mm1 of chunk c+1, so the PE never stalls on the exp (ACT)
            pend = []

            def flush_pend():
                if not pend:
                    return
                pb, pi, pexpT, pvtf = pend.pop()

                def mm2(tt, j, stop):
                    v0 = tt * H * E + j * E
                    nc.tensor.matmul(
                        accs[j // 4][32 * (j % 4):32 * (j % 4) + 4, 0:E],
                        pexpT[:, tt * H * S + 4 * j:
                              tt * H * S + 4 * j + 4],
                        pvtf[:, v0:v0 + E],
                        start=False, stop=stop,
                        skip_group_check=True,
                        tile_position=(0, 32 * (j % 4)))

                if pi < NI - 1:
                    for tt in range(2):
                        for j in range(H):
                            mm2(tt, j, False)
                else:
                    # final chunk of batch pb: drain each accumulator bank
                    # right after its own last matmul so normalization and
                    # stores overlap the remaining banks' matmuls.
                    for j in range(H):
                        mm2(0, j, False)
                    o_all = drain.tile([128, 512], F32, tag="o_all",
                                       name=f"o_all_{pb}")
                    for g in range(4):
                        for j in range(4 * g, 4 * g + 4):
                            mm2(1, j, j % 4 == 3)
                        rs = drain.tile([128, 1], F32, tag=f"rs{g}",
                                        name=f"rs{g}_{pb}")
                        nc.vector.reciprocal(rs[:], accs[g][:, D:E])
                        nc.scalar.activation(o_all[:, g * D:(g + 1) * D],
                                             accs[g][:, 0:D], AF.Copy,
                                             scale=rs[:])
                    # stores ride the SWDGE (gpsimd) path: their scattered
                    # descriptors would otherwise sit in the sync ring's
                    # outstanding window and stall the K stream right at
                    # the batch boundary
                    for j in range(4):
                        nc.gpsimd.dma_start(
                            out_d[pb, :, :].rearrange(
                                "s (g j d) -> j s g d", g=4, d=D)[j],
                            o_all[32 * j:32 * j + S, :]
                            .rearrange("p (g d) -> p g d", d=D),
                        )

            for idx, (b, i) in enumerate(chunks):
                if i == 0:
                    # flush the previous batch's pending mm2 + drain BEFORE
                    # recycling the accumulator banks (PE is strict FIFO; a
                    # later-emitted matmul can't unblock an earlier one)
                    flush_pend()
                    # 4 PSUM accumulator banks (one per group of 4 heads):
                    # rows 32j+0..4 = o[q, :] of head 4g+j; col 128 = sums.
                    accs = [psacc.tile([128, 512], F32, tag=f"acc{g}",
                                       name=f"acc{g}_{b}")
                            for g in range(4)]
                    for g in range(4):
                        nc.vector.memset(accs[g][:, 0:E], 0.0)

                if i == 1:
                    # the 4 new (current) keys, causal-masked -- emitted
                    # here (not at i==0) so it never gates the first
                    # chunk's matmuls, and never lands in the kernel tail
                    if kTn is None:
                        kTn = _transpose_cast(nc, sb, ps, xr_k, ident, "k")
                    vnaf = vnafs[b]
                    sn = ps.tile([128, 512], F32, tag="sT",
                                 name=f"sn_{b}")
                    for j in range(H):
                        c = _col(b, j)
                        nc.tensor.matmul(sn[0:S, 4 * j:4 * j + 4],
                                         kTn[:, c:c + S], qT[:, c:c + S],
                                         start=(j == 0), stop=(j == H - 1),
                                         skip_group_check=True)
                    en = expp.tile([S, H * S], F16, tag="en",
                                   name=f"en_{b}")
                    nc.scalar.activation(en[:], sn[0:S, 0:H * S], AF.Exp,
                                         scale=SCALE)
                    enm = expp.tile([S, H * S], F16, tag="enm",
                                    name=f"enm_{b}")
                    nc.vector.tensor_mul(enm[:], en[:], mask16[:])
                    corner = (enm, vnafs[b])

                issue_dma(idx + 3)
                issue_cvt(idx + 1)
                ktf, vtf = cvt_tiles.pop(idx)
                # scores for both 128-row sub-tiles share one PSUM bank
                # -> a single 128-col exp per chunk
                sT = ps.tile([128, 512], F32, tag="sT", name=f"sT_{idx}")
                for tt in range(2):
                    for j in range(H):
                        c = _col(b, j)
                        k0 = tt * H * 128 + j * 128
                        nc.tensor.matmul(
                            sT[:, tt * H * S + 4 * j:
                               tt * H * S + 4 * j + 4],
                            ktf[:, k0:k0 + 128], qT[:, c:c + S],
                            start=(tt == 0 and j == 0),
                            stop=(tt == 1 and j == H - 1),
                            skip_group_check=True)
                expT = expp.tile([128, 2 * H * S], F16, tag="expT",
                                 name=f"expT_{idx}")
                scol = b * NI + i
                nc.scalar.activation(expT[:], sT[:, 0:2 * H * S], AF.Exp,
                                     scale=ssc_sb[:, scol:scol + 1],
                                     bias=bia_sb[:, scol:scol + 1])
                flush_pend()
                if i == 1:
                    # corner accumulation, emitted after a full chunk of PE
                    # work so the PE never stalls on the corner's exp
                    cenm, cvnaf = corner
                    for j in range(H):
                        nc.tensor.matmul(
                            accs[j // 4][32 * (j % 4):32 * (j % 4) + 4,
                                         0:E],
                            cenm[:, 4 * j:4 * j + 4],
                            cvnaf[:, j * E:(j + 1) * E],
                            start=False, stop=False,
                            skip_group_check=True,
                            tile_position=(0, 32 * (j % 4)))
                pend.append((b, i, expT, vtf))
            flush_pend()

    nc.compile()
    return nc


_NC_CACHE = []


def _get_nc():
    if not _NC_CACHE:
        _NC_CACHE.append(build())
    return _NC_CACHE[0]


def _make_in_maps(q, k, v, freqs_cos, freqs_sin, cache_k, cache_v, q_norm_w,
                  k_norm_w):
    q = np.asarray(q, dtype=np.float32)
    k = np.asarray(k, dtype=np.float32)
    v = np.asarray(v, dtype=np.float32)
    cache_k = np.asarray(cache_k, dtype=np.float32)
    cache_v = np.asarray(cache_v, dtype=np.float32)
    freqs_cos = np.asarray(freqs_cos, dtype=np.float32)
    freqs_sin = np.asarray(freqs_sin, dtype=np.float32)
    q_norm_w = np.asarray(q_norm_w, dtype=np.float32)
    k_norm_w = np.asarray(k_norm_w, dtype=np.float32)

    # host-side constant marshalling (layout/dtype helpers only)
    cos_b = np.ascontiguousarray(
        np.broadcast_to(freqs_cos[None, None], (B_LOC, H, S, D // 2))
        .reshape(P, D // 2))
    sin_b = np.ascontiguousarray(
        np.broadcast_to(freqs_sin[None, None], (B_LOC, H, S, D // 2))
        .reshape(P, D // 2))
    wq_b = np.ascontiguousarray(np.broadcast_to(q_norm_w[None, :], (P, D)))
    wk_b = np.ascontiguousarray(np.broadcast_to(k_norm_w[None, :], (P, D)))
    ident = np.eye(128, dtype=np.float32)
    # mask[t, j*4+i] = 1 if query i attends new key t (i >= t), per 16 heads
    mask = (np.arange(S)[None, :] >= np.arange(S)[:, None]).astype(np.float16)
    mask = np.ascontiguousarray(np.tile(mask, (1, H)))  # [4, 64]

    # q/k packed into the [(b h s), d] preproc layout
    qp_all = np.ascontiguousarray(
        q.reshape(B, S, H, D).transpose(0, 2, 1, 3)).reshape(B, H * S, D)
    kp_all = np.ascontiguousarray(
        k.reshape(B, S, H, D).transpose(0, 2, 1, 3)).reshape(B, H * S, D)

    # --- K cache: per (b, chunk, row-pair) scales -------------------------
    # chunk rows: [2(tt), 128(r), H, D]; scale shared across (tt, h, d)
    kc = cache_k.reshape(B, NI, 2, 128, H, D)
    k_abs = np.abs(kc).max(axis=(2, 4, 5))  # [B, NI, 128]
    k_abs = np.maximum(k_abs, 1e-6)
    ckr = (127.0 / k_abs).astype(np.float32)  # [B, NI, 128]
    ktw_all = np.empty((B, NKW, D, 2 * H * 128), np.int8)
    kte_all = np.empty((B, NKE, D, 2 * H * 128), E3M4)
    ssc_all = np.empty((B, NI, 128), np.float32)
    for i in range(NI):
        blk = kc[:, i]  # [B, 2, 128, H, D]
        if KCLS[i] == 'W':
            ki = np.clip(np.round(blk * ckr[:, i, None, :, None, None]),
                         -127, 127).astype(np.int8)
            # -> [B, D, 2, H, 128] -> [B, D, 2*H*128]
            ktw_all[:, KW_IDX[i]] = (ki.transpose(0, 4, 1, 3, 2)
                                     .reshape(B, D, 2 * H * 128))
            ssc_all[:, i] = SCALE / ckr[:, i]
        else:
            ke = (blk * E3K_SCALE).astype(E3M4)
            kte_all[:, KE_IDX[i]] = (ke.transpose(0, 4, 1, 3, 2)
                                     .reshape(B, D, 2 * H * 128))
            ssc_all[:, i] = SCALE / E3K_SCALE

    # --- V cache: per (b, chunk, row-pair) INTEGER scales -----------------
    vc = cache_v.reshape(B, NI, 2, 128, H, D)
    v_abs = np.abs(vc).max(axis=(2, 4, 5))  # [B, NI, 128]
    v_abs = np.maximum(v_abs, 1e-6)
    cvr = np.clip(np.floor(127.0 / v_abs), 1, 127).astype(np.float32)
    vbw_all = np.empty((B, NVW, 128, 2 * H * E), np.int8)
    vbc_all = np.empty((B, NVC, 128, 2 * H * E), np.int8)
    bia_all = np.empty((B, NI, 128), np.float32)
    vaug = np.empty((B, 2, 128, H, E), np.int8)
    for i in range(NI):
        blk = vc[:, i]  # [B, 2, 128, H, D]
        vi = np.clip(np.round(blk * cvr[:, i, None, :, None, None]),
                     -127, 127).astype(np.int8)
        vaug[:, :, :, :, 0:D] = vi
        vaug[:, :, :, :, D] = cvr[:, i, None, :, None]
        # [B, 2, 128, H, E] -> [B, 128, 2, H, E] -> [B, 128, 2*H*E]
        packed = (vaug.transpose(0, 2, 1, 3, 4).reshape(B, 128, 2 * H * E))
        if VCLS[i] == 'W':
            vbw_all[:, VW_IDX[i]] = packed
        else:
            vbc_all[:, VC_IDX[i]] = packed
        bia_all[:, i] = -np.log(cvr[:, i])

    # new-token V with the exact ones column (scale 1)
    vna_all = np.empty((B, S, H, E), np.float16)
    vna_all[:, :, :, 0:D] = v.reshape(B, S, H, D)
    vna_all[:, :, :, D] = 1.0

    in_maps = []
    for ci in range(N_CORES):
        bs = slice(ci * B_LOC, (ci + 1) * B_LOC)
        # ssc/bia: [128(row), B_LOC*NI] with col = b*NI + i
        ssc = np.ascontiguousarray(
            ssc_all[bs].reshape(B_LOC * NI, 128).T)
        bia = np.ascontiguousarray(
            bia_all[bs].reshape(B_LOC * NI, 128).T)
        im = {
            "qp": np.ascontiguousarray(qp_all[bs]).reshape(P, D),
            "kp": np.ascontiguousarray(kp_all[bs]).reshape(P, D),
            "vna": np.ascontiguousarray(vna_all[bs]).reshape(B_LOC, S, H * E),
            "ssc": ssc, "bia": bia,
            "cos_b": cos_b, "sin_b": sin_b, "wq_b": wq_b, "wk_b": wk_b,
            "ident": ident, "mask": mask,
        }
        if NKW:
            im["ktw"] = np.ascontiguousarray(ktw_all[bs])
        if NKE:
            im["kte"] = np.ascontiguousarray(kte_all[bs])
        if NVW:
            im["vbw"] = np.ascontiguousarray(vbw_all[bs])
        if NVC:
            im["vbc"] = np.ascontiguousarray(vbc_all[bs])
        in_maps.append(im)
    return in_maps


def run(q, k, v, freqs_cos, freqs_sin, cache_k, cache_v, q_norm_w, k_norm_w,
        trace=False):
    in_maps = _make_in_maps(q, k, v, freqs_cos, freqs_sin, cache_k, cache_v,
                            q_norm_w, k_norm_w)
    nc = _get_nc()
    res = run_bass_kernel_spmd(nc, in_maps, list(range(N_CORES)), trace=trace)
    out = np.concatenate([res.results[i]["out"] for i in range(N_CORES)],
                         axis=0)
    return out.reshape(B, S, DIM), res


def kernel(q, k, v, freqs_cos, freqs_sin, cache_k, cache_v, q_norm_w,
           k_norm_w):
    out, _ = run(q, k, v, freqs_cos, freqs_sin, cache_k, cache_v, q_norm_w,
                 k_norm_w)
    return out
